# revision 39
# baseline (speedup 1.0000x reference)
"""MultiHeadChannelAttention Bass kernel for 8 Trainium2 NeuronCores.

Problem (hardcoded shapes): x (2, 512, 64, 32) fp32; Wq/Wk/Wv/Wfc (512, 512);
biases (512,). Reference math per batch b, with X = x[b].reshape(2048, 512):
  Q = X Wq^T + bq ; K = X Wk^T + bk ; V = X Wv^T + bv   (heads of 64 dims)
  out = softmax(QK^T/8) V  (per head), concat heads, @ Wfc^T + bfc

Sharding: 8 cores = 2 batches x 4 token-blocks of 512 tokens. Each core
computes K/V for all 2048 tokens of its batch (4x redundant), Q/attention/fc
only for its 512-token block. No cross-core communication.

fp8 (e4m3) DoubleRow perf mode carries all the heavy math at 2x PE rate:
  - projections contract 256 channels per pass (128 partitions x 2-row
    interleave). Weights are pre-scaled by S=32 on the host so they sit in
    e4m3's normal range (raw std 0.02 is subnormal); Wfc absorbs 1/S.
    Residual passes (fp8 of the cast error) recover most of the fp8
    quantization: Q/K use 2 passes (x + x-residual), V uses 3 (+ W-residual).
  - scores: projection outputs stored fp8 and DMA-restructured to a
    [32p, (head, half, tok)] layout so each head's dk=64 contraction runs as
    32 partitions x 2 rows at 0.5 cyc/col. Pair 0 runs its first two steps
    straight off the [128, tok] layout (64-wide contraction, half rate) so
    the first exp never waits on the restructure round-trip.
  - attnV: exp writes fp8 directly; V carries a ones column so the softmax
    denominator falls out of the same matmul; two key-tiles (128 keys each)
    contract per pass via the 2-row interleave.

ScalarE (ACT) runs only the 64 [128,1024] exps -- it is the pacing engine
(~69 us busy). Softmax reciprocals run on DVE (reciprocal_approx_accurate);
the denominator broadcast over dk stays a PE ones-matmul. bv is folded into
the fc bias on host (softmax rows sum to 1). The fc tail stays bf16.
"""

import numpy as np
import ml_dtypes

N_CORES = 8
B, C, N_TOK, TB = 2, 512, 2048, 512
HEADS, DK = 8, 64
NCH = C // 128  # channel chunks (4)
NPC = NCH // 2  # chunk pairs for fp8 DoubleRow projections (2)
NJT = N_TOK // 128  # key-token tiles (16)
NT = NJT // 2  # key-tile pairs per attnV pass (8)
NTT = TB // 128  # fc token tiles (4)
WSCALE = 32.0  # host pre-scale keeping fp8 weights out of subnormals

_CACHE = {}


def _install_tile_drain_patch():
    """The end-of-kernel Tile drain can carry several sem waits; this
    walrus build allows one wait per non-EVSEM instruction. Split the
    waits across a chain of drains."""
    import bass_rust
    from concourse import tile as _tile
    from concourse.vector_clock import ScopedClock

    if getattr(_tile.TileContext, "_drain_patch_installed", False):
        return

    def _patched(self, tick_clock, wait_clock):
        nc = self.nc
        drain_inst = nc.sync.drain()
        wait_clock.add_sem_waits(
            drain_inst.ins, ScopedClock({None: tick_clock.global_clock})
        )
        si = drain_inst.ins.sync_info
        if si is not None and len(si.on_wait) > 1:
            waits = list(si.on_wait)
            drain_inst.ins.sync_info = bass_rust.SyncInfo(
                on_wait=[waits[0]], on_update=list(si.on_update)
            )
            for w in waits[1:]:
                extra = nc.sync.drain()
                extra.ins.sync_info = bass_rust.SyncInfo(on_wait=[w], on_update=[])
        nc.all_engine_barrier()
        assert self.sems is not None
        popped = nc._tile_sem_poison_stack.pop()
        assert popped is self._sem_poison
        nc.clear_and_free_semaphores(list(self.sems.allocated().values()))
        nc.all_engine_barrier()

    _tile.TileContext._drain_and_barrier = _patched
    _tile.TileContext._drain_patch_installed = True


def _split_multi_waits(nc):
    """This walrus build accepts one sync wait per instruction (two on
    EVSEM). Tile can attach two; move extras onto preceding NOPs."""
    import concourse.mybir as mybir

    for f in nc.m.functions:
        for bb in f.blocks:
            out = []
            changed = False
            for ins in bb.instructions:
                si = ins.sync_info
                limit = 2 if isinstance(ins, mybir.InstEventSemaphore) else 1
                if si is not None and len(si.on_wait) > limit:
                    waits = list(si.on_wait)
                    keep = waits[-limit:]
                    for i, w in enumerate(waits[:-limit]):
                        nop = mybir.InstNoOp(
                            name=f"{ins.name}_w{i}",
                            engine=ins.engine,
                            sync_info=mybir.SyncInfo(on_wait=[w], on_update=[]),
                            bass_nofuse=True,
                        )
                        nc.register_instruction(nop, overwrite=True)
                        out.append(nop)
                    ins.sync_info = mybir.SyncInfo(
                        on_wait=keep, on_update=list(si.on_update)
                    )
                    changed = True
                out.append(ins)
            if changed:
                bb.instructions = out


def _build():
    import concourse.bass as bass
    import concourse.mybir as mybir
    import concourse.tile as tile
    from concourse.bass import ts

    dt = mybir.dt
    f32, bf16, f8 = dt.float32, dt.bfloat16, dt.float8e4
    Exp = mybir.ActivationFunctionType.Exp
    Ln = mybir.ActivationFunctionType.Ln
    Copy = mybir.ActivationFunctionType.Copy
    DR = mybir.MatmulPerfMode.DoubleRow
    EXP_SCALE = 0.125 / (WSCALE * WSCALE)

    nc = bass.Bass()
    # fp8 operands, host pre-interleaved: chunk-pair axes are (pc, i) with
    # input channel c = 128*(2*pc+i)+partition. *r = fp8 residual terms.
    xf8_d = nc.dram_tensor("xf8", [128, NPC, 2, N_TOK], f8, kind="ExternalInput")
    xr8_d = nc.dram_tensor("xr8", [128, NPC, 2, N_TOK], f8, kind="ExternalInput")
    xq8_d = nc.dram_tensor("xq8", [128, NPC, 2, TB], f8, kind="ExternalInput")
    xqr8_d = nc.dram_tensor("xqr8", [128, NPC, 2, TB], f8, kind="ExternalInput")
    wq2_d = nc.dram_tensor("wq2", [128, NPC, 2, C], f8, kind="ExternalInput")
    wk2_d = nc.dram_tensor("wk2", [128, NPC, 2, C], f8, kind="ExternalInput")
    wv2_d = nc.dram_tensor("wv2", [128, NPC, 2, C], f8, kind="ExternalInput")
    wvr2_d = nc.dram_tensor("wvr2", [128, NPC, 2, C], f8, kind="ExternalInput")
    wfT_d = nc.dram_tensor("wfT", [128, NCH * C], bf16, kind="ExternalInput")
    bias_d = nc.dram_tensor("bias", [128, 2 * NCH], f32, kind="ExternalInput")
    bfc_d = nc.dram_tensor("bfc", [1, C], bf16, kind="ExternalInput")
    out_d = nc.dram_tensor("out", [TB, C], bf16, kind="ExternalOutput")

    with tile.TileContext(nc) as tc:
        with (
            tc.tile_pool(name="wp", bufs=1) as wp,
            tc.tile_pool(name="data", bufs=1) as data,
            tc.tile_pool(name="ep", bufs=6) as ep,
            tc.tile_pool(name="np_", bufs=2) as npool,
            tc.tile_pool(name="scp", bufs=2, space=bass.MemorySpace.PSUM) as scp,
            tc.tile_pool(name="ap_", bufs=1, space=bass.MemorySpace.PSUM) as apool,
            tc.tile_pool(name="aux", bufs=2, space=bass.MemorySpace.PSUM) as aux,
        ):
            # ---- constants / weights ----
            wq2 = wp.tile([128, NPC, 2, C], f8, tag="wq", name="wq2")
            wk2 = wp.tile([128, NPC, 2, C], f8, tag="wk", name="wk2")
            wv2 = wp.tile([128, NPC, 2, C], f8, tag="wv", name="wv2")
            wvr2 = wp.tile([128, NPC, 2, C], f8, tag="wvr", name="wvr2")
            wf_all = wp.tile([128, NCH * C], bf16, tag="wf", name="wf_all")
            wf = [wf_all[:, ts(c, C)] for c in range(NCH)]
            bias_all = wp.tile([128, 2 * NCH], f32, tag="bias", name="bias_all")
            bqt = [bias_all[:, d : d + 1] for d in range(NCH)]
            bkt = [bias_all[:, NCH + d : NCH + d + 1] for d in range(NCH)]
            bfct = wp.tile([1, C], bf16, tag="bfct", name="bfct")
            ones_t = wp.tile([128, TB], bf16, tag="ones", name="ones_t")
            nc.vector.memset(ones_t[:], 1.0)
            ones_f = wp.tile([128, 64], f32, tag="onesf", name="ones_f")
            nc.vector.memset(ones_f[:], 1.0)
            zeros_t = wp.tile([1, 128], bf16, tag="zeros", name="zeros_t")
            nc.vector.memset(zeros_t[:], 0.0)

            # preload the Exp ACT table during the input-DMA window
            actwarm = npool.tile([1, 8], f32, tag="actw", name="actwarm")
            nc.scalar.activation(out=actwarm[:], in_=ones_f[0:1, 0:8], func=Exp)

            # PE warmup: dummy matmuls ramp the HAM activity monitor through
            # the input-load window so the projections run at speed
            warm = aux.tile([128, TB], f32, tag="aux", name="warm")
            for r in range(8):
                nc.tensor.matmul(
                    warm[:], ones_t[0:1, 0:128], ones_t[0:1, :],
                    start=(r == 0), stop=(r == 7),
                )

            # ---- activations in ----
            xf8 = data.tile([128, NPC, 2, N_TOK], f8, tag="xf8", name="xf8")
            xr8 = data.tile([128, NPC, 2, N_TOK], f8, tag="xr8", name="xr8")
            xq8 = data.tile([128, NPC, 2, TB], f8, tag="xq8", name="xq8")
            xqr8 = data.tile([128, NPC, 2, TB], f8, tag="xqr8", name="xqr8")

            # ---- input DMAs. Time-to-first-exp is bound by bias+wk2+xf8(jb0)
            # +xq8+wq2 (main terms only -- pair 0's d=0 projections skip the
            # residual pass). SP/Pool queues then carry the K/Q restructure
            # stream; residual/bulk tensors ride ACT (idle until first exp)
            # and Pool ----
            nc.sync.dma_start(out=bias_all[:], in_=bias_d[:])
            nc.scalar.dma_start(out=wq2[:], in_=wq2_d[:])
            nc.sync.dma_start(out=wk2[:], in_=wk2_d[:])
            nc.scalar.dma_start(out=xq8[:], in_=xq8_d[:])
            nc.sync.dma_start(out=xf8[:, :, :, ts(0, TB)], in_=xf8_d[:, :, :, ts(0, TB)])
            nc.gpsimd.dma_start(out=wv2[:], in_=wv2_d[:])
            nc.scalar.dma_start(
                out=xf8[:, :, :, TB:N_TOK], in_=xf8_d[:, :, :, TB:N_TOK]
            )
            nc.gpsimd.dma_start(out=xr8[:], in_=xr8_d[:])
            nc.gpsimd.dma_start(out=wvr2[:], in_=wvr2_d[:])
            nc.gpsimd.dma_start(out=xqr8[:], in_=xqr8_d[:])
            nc.gpsimd.dma_start(out=wf_all[:], in_=wfT_d[:])
            nc.gpsimd.dma_start(out=bfct[:], in_=bfc_d[:])

            # ---- persistent intermediates ----
            kt8 = [
                data.tile([128, N_TOK], f8, tag=f"kt8_{d}", name=f"kt8_{d}")
                for d in range(NCH)
            ]
            qt8 = [
                data.tile([128, TB], f8, tag=f"qt8_{d}", name=f"qt8_{d}")
                for d in range(NCH)
            ]
            # DoubleRow-layout K/Q: [32p, head hh, half i, tok]; dk = 32*i + p
            ktd = [
                data.tile([32, 2, 2, N_TOK], f8, tag=f"ktd{d}", name=f"ktd{d}")
                for d in range(NCH)
            ]
            qtd = [
                data.tile([32, 2, 2, TB], f8, tag=f"qtd{d}", name=f"qtd{d}")
                for d in range(NCH)
            ]
            # V pairs: [128p tok, half i, head, dk+ones+pad] -- the pad
            # column keeps the DoubleRow Ldweights length even
            vpad2 = [
                data.tile([128, 2, HEADS, DK + 2], f8, tag=f"vp{t}", name=f"vp{t}")
                for t in range(NT)
            ]
            att = [
                data.tile([128, TB], bf16, tag=f"att{c}", name=f"att{c}")
                for c in range(NCH)
            ]
            for t in range(NT):
                nc.vector.memset(vpad2[t][:, :, :, DK : DK + 1], 1.0)
                nc.vector.memset(vpad2[t][:, :, :, DK + 1 : DK + 2], 0.0)

            def restructure(dst, src, jsl):
                """[128, tok] fp8 -> [32, head, half, tok] via SBUF-SBUF DMA;
                hh=0 slabs on the SP queue, hh=1 on the Pool queue."""
                for hh in range(2):
                    eng = nc.sync if hh == 0 else nc.gpsimd
                    for i in range(2):
                        eng.dma_start(
                            out=dst[:, hh, i, jsl],
                            in_=src[hh * 64 + i * 32 : hh * 64 + (i + 1) * 32, jsl],
                        )

            def proj_q(d):
                """Q^T d-tile (128 chans = heads 2d, 2d+1) + bias -> fp8 +
                restructure. d=0 runs main-term only and skips the
                restructure (its scores read qt8 directly)."""
                qp = aux.tile([128, TB], f32, tag="aux", name=f"qp{d}")
                nterm = 1 if d == 0 else 2
                for pc in range(NPC):
                    nc.tensor.matmul(
                        qp[:], wq2[:, pc, :, ts(d, 128)], xq8[:, pc],
                        start=(pc == 0), stop=(pc == NPC - 1 and nterm == 1),
                        perf_mode=DR,
                    )
                if nterm == 2:
                    for pc in range(NPC):
                        nc.tensor.matmul(
                            qp[:], wq2[:, pc, :, ts(d, 128)], xqr8[:, pc],
                            start=False, stop=(pc == NPC - 1), perf_mode=DR,
                        )
                nc.vector.tensor_scalar_add(out=qt8[d][:], in0=qp[:], scalar1=bqt[d])
                restructure(qtd[d], qt8[d], slice(None))

            def proj_k(d, jb):
                """K^T d-tile, token block jb + bias -> fp8 (+ restructure)."""
                kp = aux.tile([128, TB], f32, tag="aux", name=f"kp{d}_{jb}")
                nterm = 1 if d == 0 else 2
                for pc in range(NPC):
                    nc.tensor.matmul(
                        kp[:], wk2[:, pc, :, ts(d, 128)], xf8[:, pc, :, ts(jb, TB)],
                        start=(pc == 0), stop=(pc == NPC - 1 and nterm == 1),
                        perf_mode=DR,
                    )
                if nterm == 2:
                    for pc in range(NPC):
                        nc.tensor.matmul(
                            kp[:], wk2[:, pc, :, ts(d, 128)], xr8[:, pc, :, ts(jb, TB)],
                            start=False, stop=(pc == NPC - 1), perf_mode=DR,
                        )
                nc.vector.tensor_scalar_add(
                    out=kt8[d][:, ts(jb, TB)], in0=kp[:], scalar1=bkt[d]
                )
                restructure(ktd[d], kt8[d], ts(jb, TB))

            def proj_v(j):
                """V j-tile -> vpad2[j//2] half j%2 (fp8, ones col preset).
                DoubleRow passes: x*w + xres*w (+ x*wres for j>=4; the first
                two t-steps of pair 0 are PE-tight, so their tiles drop the
                W-residual pass)."""
                vp = aux.tile([128, C], f32, tag="aux", name=f"vpp{j}")
                jsl = ts(j, 128)
                three = j >= 4
                for pc in range(NPC):
                    nc.tensor.matmul(
                        vp[:], xf8[:, pc, :, jsl], wv2[:, pc],
                        start=(pc == 0), stop=False, perf_mode=DR,
                    )
                for pc in range(NPC):
                    nc.tensor.matmul(
                        vp[:], xr8[:, pc, :, jsl], wv2[:, pc],
                        start=False, stop=(pc == NPC - 1 and not three),
                        perf_mode=DR,
                    )
                if three:
                    for pc in range(NPC):
                        nc.tensor.matmul(
                            vp[:], xf8[:, pc, :, jsl], wvr2[:, pc],
                            start=False, stop=(pc == NPC - 1), perf_mode=DR,
                        )
                nc.vector.tensor_copy(
                    out=vpad2[j // 2][:, j % 2, :, 0:DK],
                    in_=vp[:].rearrange("p (h d) -> p h d", h=HEADS),
                )

            def scores_exp(p, t, hh):
                """Scores for head 2p+hh over j-tiles 2t,2t+1 -> exp -> fp8 e.
                Pair 0's first two steps contract dk=64 directly off kt8/qt8
                (half PE rate, zero restructure latency); everything else
                uses the 32x2 DoubleRow layout at full fp8 rate."""
                sc = scp.tile([128, 2, TB], f32, tag="sc", name=f"sc{p}_{t}_{hh}")
                for i2 in range(2):
                    if p == 0 and t < 2:
                        nc.tensor.matmul(
                            sc[:, i2, :],
                            kt8[0][ts(hh, 64), ts(2 * t + i2, 128)],
                            qt8[0][ts(hh, 64), :],
                        )
                    else:
                        nc.tensor.matmul(
                            sc[:, i2, :],
                            ktd[p][:, hh, :, ts(2 * t + i2, 128)],
                            qtd[p][:, hh],
                            perf_mode=DR,
                        )
                e = ep.tile([128, 2, TB], f8, tag="e", name=f"e{p}_{t}_{hh}")
                nc.scalar.activation(out=e[:], in_=sc[:], func=Exp, scale=EXP_SCALE)
                return e

            def attn_v(p, t, hh, e, a):
                nc.tensor.matmul(
                    a[:], vpad2[t][:, :, 2 * p + hh, :], e[:],
                    start=(t == 0), stop=(t == NT - 1), perf_mode=DR,
                )

            def normalize(pp, a_sb, hh, rb=None):
                """Softmax normalization for pair pp's head hh. Denominator
                reciprocal on DVE (approx_accurate, ~2ulp); broadcast over
                the 64 dk partitions via a PE ones-matmul; scale on DVE."""
                if rb is None:
                    rb = aux.tile([64, TB], f32, tag="aux", name=f"rb{pp}_{hh}")[:]
                rcp = npool.tile([65, TB], f32, tag="rcp", bufs=4, name=f"rcp{pp}_{hh}")
                nc.vector.reciprocal(out=rcp[64:65, :], in_=a_sb[64:65, :])
                nc.tensor.matmul(rb, ones_f[64:65, :], rcp[64:65, :])
                nc.vector.tensor_mul(
                    out=att[pp][ts(hh, 64), :], in0=a_sb[0:64, :], in1=rb
                )

            def fc_prefill(tt, fp):
                nc.tensor.matmul(
                    fp, ones_t[0:1, 0:128], bfct[:], start=True, stop=False
                )
                for c in range(NCH - 1):
                    nc.tensor.matmul(
                        fp, att[c][:, ts(tt, 128)], wf[c],
                        start=False, stop=False,
                    )

            # ---- main pipeline ----
            fcs = {}
            proj_q(0)
            proj_k(0, 0)
            prev = None  # previous pair's SBUF accumulator copies
            pending = None  # previous pair's boundary work, run at (p, t=0)
            for p in range(NCH):  # head pair p = heads 2p, 2p+1
                a0 = apool.tile([DK + 2, TB], f32, tag="a0", name=f"a0_{p}")
                a1 = apool.tile([DK + 2, TB], f32, tag="a1", name=f"a1_{p}")
                es = {}
                for t in range(NT):
                    # pair seam: emit a two-step scores runway before the
                    # previous pair's boundary leftovers, so the exp stream
                    # never waits on the PE's in-order queue
                    if not (p >= 1 and t == 1):
                        es[(t, 0)] = scores_exp(p, t, 0)
                        es[(t, 1)] = scores_exp(p, t, 1)
                    if p >= 1 and t == 0:
                        es[(1, 0)] = scores_exp(p, 1, 0)
                        es[(1, 1)] = scores_exp(p, 1, 1)
                        prev = pending()
                        pending = None
                    # pair 0: remaining K blocks paced with the xf8 DMA and
                    # kept off the PE-tight t=1 step; the restructure lands
                    # two steps before each block's first scores
                    if p == 0 and t in (0, 2, 3):
                        proj_k(0, 1 if t == 0 else t)
                    # next pair's K/Q projections, spread mid-pair so every
                    # restructure DMA lands before the pair boundary (pair
                    # 0 is PE-tight, so its share shifts one step later and
                    # projk(1,3) rides the seam leftover block)
                    if p == 0:
                        if t == 4:
                            proj_q(1)
                        elif t >= 5:
                            proj_k(1, t - 5)
                    elif p < NCH - 1:
                        if t == 3:
                            proj_q(p + 1)
                        elif t >= 4:
                            proj_k(p + 1, t - 4)
                    else:
                        # last pair: fc pre-accumulation (bias + chunks 0..2)
                        # as PE filler; fp0/fp1 in the freed aux slots
                        if t in (3, 4):
                            fcs[t - 3] = aux.tile(
                                [128, C], f32, tag="aux", name=f"fp{t - 3}"
                            )[:]
                            fc_prefill(t - 3, fcs[t - 3])
                    # previous pair's normalization, deferred into this pair
                    if prev is not None and t in (1, 2):
                        hh = t - 1
                        normalize(p - 1, prev[hh], hh)
                    # attnV: pair 0 lags one t-step so the V-operand DMAs and
                    # projections stay off the exp-stream critical path
                    tv = t - 1 if p == 0 else t
                    if 0 <= tv:
                        if p == 0:
                            proj_v(2 * tv)
                            proj_v(2 * tv + 1)
                        attn_v(p, tv, 0, es.pop((tv, 0)), a0)
                        attn_v(p, tv, 1, es.pop((tv, 1)), a1)

                def boundary(p=p, a0=a0, a1=a1, es=es):
                    if p == 0:
                        proj_v(NJT - 2)
                        proj_v(NJT - 1)
                        attn_v(p, NT - 1, 0, es.pop((NT - 1, 0)), a0)
                        attn_v(p, NT - 1, 1, es.pop((NT - 1, 1)), a1)
                        proj_k(1, 3)
                    # evacuate accumulators to SBUF (DVE) so the banks free
                    a_sb0 = npool.tile(
                        [DK + 1, TB], f32, tag="asb", bufs=4, name=f"asb0_{p}"
                    )
                    a_sb1 = npool.tile(
                        [DK + 1, TB], f32, tag="asb", bufs=4, name=f"asb1_{p}"
                    )
                    nc.vector.tensor_copy(out=a_sb0[:], in_=a0[0 : DK + 1, :])
                    nc.vector.tensor_copy(out=a_sb1[:], in_=a1[0 : DK + 1, :])
                    return (a_sb0, a_sb1)

                if p < NCH - 1:
                    pending = boundary
                else:
                    # last pair: normalize straight out of PSUM at the tail
                    prev = (a0, a1)

            # ---- tail: only the final fc chunk waits on normalize(3).
            # a0/a1 stay live (PSUM-direct reciprocal + scale); head 6's
            # reciprocal runs on ACT (idle now; Ln/Exp/Copy share the loaded
            # table), head 7's on DVE, in parallel. The freed a0/a1 slots
            # take the rb broadcasts; fp2/fp3 take the scores slots ----
            a0, a1 = prev
            fp2 = scp.tile([128, 2, C], f32, tag="sc", name="fp2")
            fp3 = scp.tile([128, 2, C], f32, tag="sc", name="fp3")
            fcs[2], fcs[3] = fp2[:, 0, :], fp3[:, 0, :]
            fc_prefill(2, fcs[2])
            fc_prefill(3, fcs[3])

            lnt0 = npool.tile([65, TB], f32, tag="lnt", bufs=4, name="lnt3_0")
            rcp0 = npool.tile([65, TB], f32, tag="rcp", bufs=4, name="rcp3_0")
            lnt1 = npool.tile([65, TB], f32, tag="lnt", bufs=4, name="lnt3_1")
            rcp1 = npool.tile([65, TB], f32, tag="rcp", bufs=4, name="rcp3_1")
            asb0 = npool.tile([DK + 1, TB], f32, tag="asb", bufs=4, name="asb3_0")
            asb1 = npool.tile([DK + 1, TB], f32, tag="asb", bufs=4, name="asb3_1")
            nc.scalar.activation(out=lnt0[64:65, :], in_=a0[64:65, :], func=Ln)
            nc.scalar.activation(
                out=rcp0[64:65, :], in_=lnt0[64:65, :], func=Exp, scale=-1.0
            )
            nc.vector.reciprocal(out=rcp1[64:65, :], in_=a1[64:65, :])
            nc.scalar.activation(out=asb0[:], in_=a0[0 : DK + 1, :], func=Copy)
            nc.vector.tensor_copy(out=asb1[:], in_=a1[0 : DK + 1, :])
            # zero-contribution fillers keep the PE p-state up through the
            # reciprocal wait so the final fc runs at speed
            for r in range(2):
                nc.tensor.matmul(
                    fcs[2], zeros_t[:], ones_t[0:1, :], start=False, stop=False
                )
            rb0 = apool.tile([64, TB], f32, tag="a0", name="rb3_0")
            rb1 = apool.tile([64, TB], f32, tag="a1", name="rb3_1")
            nc.tensor.matmul(rb0[:], ones_f[64:65, :], rcp0[64:65, :])
            nc.vector.tensor_mul(
                out=att[3][0:64, :], in0=asb0[0:64, :], in1=rb0[:]
            )
            nc.tensor.matmul(rb1[:], ones_f[64:65, :], rcp1[64:65, :])
            nc.vector.tensor_mul(
                out=att[3][64:128, :], in0=asb1[0:64, :], in1=rb1[:]
            )

            # final fc split by head: the head-6 halves run as soon as the
            # first scale lands, in parallel with head 7's normalization
            for tt in range(NTT):
                nc.tensor.matmul(
                    fcs[tt], att[3][0:64, ts(tt, 128)], wf[3][0:64, :],
                    start=False, stop=False,
                )
            for tt in range(NTT):
                nc.tensor.matmul(
                    fcs[tt], att[3][64:128, ts(tt, 128)], wf[3][64:128, :],
                    start=False, stop=True,
                )
                # evacuate on ACT (idle at the tail; Copy shares Exp's table)
                ot = npool.tile([128, C], bf16, tag="ot", bufs=4, name=f"ot{tt}")
                nc.scalar.activation(out=ot[:], in_=fcs[tt], func=Copy)
                (nc.sync if tt % 2 == 0 else nc.gpsimd).dma_start(
                    out=out_d[ts(tt, 128), :], in_=ot[:]
                )

    _split_multi_waits(nc)
    nc.finalize()
    return nc


def get_nc():
    if "nc" not in _CACHE:
        _install_tile_drain_patch()
        _CACHE["nc"] = _build()
    return _CACHE["nc"]


def make_in_maps(x, Wq, bq, Wk, bk, Wv, bv, Wfc, bfc):
    bf = ml_dtypes.bfloat16
    f8 = ml_dtypes.float8_e4m3
    x = np.asarray(x, np.float32)
    Wq, Wk, Wv, Wfc = (np.asarray(w, np.float32) for w in (Wq, Wk, Wv, Wfc))
    bq, bk, bv, bfc = (np.asarray(v, np.float32) for v in (bq, bk, bv, bfc))
    S = np.float32(WSCALE)

    def interleave(wT):
        # [C, cols] -> [128, NCH*cols] with chunk c at columns [c*cols:...]
        cols = wT.shape[1]
        return np.ascontiguousarray(
            wT.reshape(NCH, 128, cols).transpose(1, 0, 2).reshape(128, NCH * cols)
        )

    def dr_pack(m):
        # [C, cols] -> [128, NPC, 2, cols]; input channel 128*(2pc+i)+p
        cols = m.shape[1]
        return np.ascontiguousarray(m.reshape(NPC, 2, 128, cols).transpose(2, 0, 1, 3))

    def split8(m):
        hi = m.astype(f8)
        lo = (m - hi.astype(np.float32)).astype(f8)
        return hi, lo

    bfc_folded = (Wfc @ bv + bfc).reshape(1, C).astype(bf)
    wq8, _ = split8(np.ascontiguousarray(Wq.T) * S)
    wk8, _ = split8(np.ascontiguousarray(Wk.T) * S)
    wv8, wvr8 = split8(np.ascontiguousarray(Wv.T) * S)
    wq2 = dr_pack(wq8)
    wk2 = dr_pack(wk8)
    wv2 = dr_pack(wv8)
    wvr2 = dr_pack(wvr8)
    wfT = interleave(np.ascontiguousarray((Wfc / S).T).astype(bf))
    bias_c = (
        np.concatenate([bq.reshape(NCH, 128).T, bk.reshape(NCH, 128).T], axis=1) * S
    ).astype(np.float32)

    in_maps = []
    for core in range(N_CORES):
        b, t = divmod(core, N_TOK // TB)
        XT = np.ascontiguousarray(x[b].reshape(N_TOK, C).T)
        x8, xr8 = split8(XT)
        XQ = np.ascontiguousarray(XT[:, t * TB : (t + 1) * TB])
        xq8, xqr8 = split8(XQ)
        in_maps.append(
            {
                "xf8": dr_pack(x8.astype(f8)),
                "xr8": dr_pack(xr8.astype(f8)),
                "xq8": dr_pack(xq8.astype(f8)),
                "xqr8": dr_pack(xqr8.astype(f8)),
                "wq2": wq2,
                "wk2": wk2,
                "wv2": wv2,
                "wvr2": wvr2,
                "wfT": wfT,
                "bias": bias_c,
                "bfc": bfc_folded,
            }
        )
    return in_maps


def assemble(outs):
    """outs: list of 8 dicts with 'out' (512, 512) -> (2, 512, 64, 32)."""
    per_batch = [
        np.concatenate([outs[b * 4 + t]["out"] for t in range(4)], axis=0)
        for b in range(B)
    ]
    return np.stack(per_batch).reshape(B, C, 64, 32).astype(np.float32)


def kernel(**inputs):
    from concourse.bass_utils import run_bass_kernel_spmd

    nc = get_nc()
    in_maps = make_in_maps(**inputs)
    res = run_bass_kernel_spmd(nc, in_maps, list(range(N_CORES)))
    return assemble(res.results)


# revision 40
# speedup vs baseline: 1.1704x; 1.1704x over previous
"""MultiHeadChannelAttention Bass kernel for 8 Trainium2 NeuronCores.

Problem (hardcoded shapes): x (2, 512, 64, 32) fp32; Wq/Wk/Wv/Wfc (512, 512);
biases (512,). Reference math per batch b, with X = x[b].reshape(2048, 512):
  Q = X Wq^T + bq ; K = X Wk^T + bk ; V = X Wv^T + bv   (heads of 64 dims)
  out = softmax(QK^T/8) V  (per head), concat heads, @ Wfc^T + bfc

Sharding: 8 cores = 2 batches x 4 token-blocks of 512 tokens. Each core
computes K/V for all 2048 tokens of its batch (4x redundant), Q/attention/fc
only for its 512-token block. No cross-core communication.

On this hardware a DoubleRow fp8 matmul streams 1 col/cycle like bf16 but
contracts 2 rows per partition, so fp8 pays off exactly where it doubles
the contraction per pass:
  - Q/K projections: x and Wq/Wk pre-cast to fp8 (weights scaled by S=32 on
    the host to clear e4m3's subnormal range; Wfc absorbs 1/S), 256-channel
    contraction per pass -- 2 passes instead of bf16's 4.
  - attnV: two key-tiles (128 keys each) contract per pass; exp writes fp8
    directly; V carries a ones column so the softmax denominator falls out
    of the same matmul (V itself is projected in bf16 for accuracy and cast
    to fp8 on the PSUM->SBUF copy).
Scores stay at the 64-wide-contraction layout straight off the fp8
projection outputs (the PE overlaps the paired stationary loads, which
beats the 32x2 DoubleRow restructure measured on hardware).

ScalarE (ACT) runs only the 64 [128,1024] exps -- it is the pacing engine
(~69 us busy). Softmax reciprocals run on DVE; the denominator broadcast
over dk is a PE ones-matmul. bv is folded into the fc bias on host
(softmax rows sum to 1). The fc tail stays bf16; output ships as bf16.
"""

import numpy as np
import ml_dtypes

N_CORES = 8
B, C, N_TOK, TB = 2, 512, 2048, 512
HEADS, DK = 8, 64
NCH = C // 128  # channel chunks (4)
NPC = NCH // 2  # chunk pairs for fp8 DoubleRow projections (2)
NJT = N_TOK // 128  # key-token tiles (16)
NT = NJT // 2  # key-tile pairs per attnV pass (8)
NTT = TB // 128  # fc token tiles (4)
WSCALE = 32.0  # host pre-scale keeping fp8 weights out of subnormals

_CACHE = {}


def _install_tile_drain_patch():
    """The end-of-kernel Tile drain can carry several sem waits; this
    walrus build allows one wait per non-EVSEM instruction. Split the
    waits across a chain of drains."""
    import bass_rust
    from concourse import tile as _tile
    from concourse.vector_clock import ScopedClock

    if getattr(_tile.TileContext, "_drain_patch_installed", False):
        return

    def _patched(self, tick_clock, wait_clock):
        nc = self.nc
        drain_inst = nc.sync.drain()
        wait_clock.add_sem_waits(
            drain_inst.ins, ScopedClock({None: tick_clock.global_clock})
        )
        si = drain_inst.ins.sync_info
        if si is not None and len(si.on_wait) > 1:
            waits = list(si.on_wait)
            drain_inst.ins.sync_info = bass_rust.SyncInfo(
                on_wait=[waits[0]], on_update=list(si.on_update)
            )
            for w in waits[1:]:
                extra = nc.sync.drain()
                extra.ins.sync_info = bass_rust.SyncInfo(on_wait=[w], on_update=[])
        nc.all_engine_barrier()
        assert self.sems is not None
        popped = nc._tile_sem_poison_stack.pop()
        assert popped is self._sem_poison
        nc.clear_and_free_semaphores(list(self.sems.allocated().values()))
        nc.all_engine_barrier()

    _tile.TileContext._drain_and_barrier = _patched
    _tile.TileContext._drain_patch_installed = True


def _split_multi_waits(nc):
    """This walrus build accepts one sync wait per instruction (two on
    EVSEM). Tile can attach two; move extras onto preceding NOPs."""
    import concourse.mybir as mybir

    for f in nc.m.functions:
        for bb in f.blocks:
            out = []
            changed = False
            for ins in bb.instructions:
                si = ins.sync_info
                limit = 2 if isinstance(ins, mybir.InstEventSemaphore) else 1
                if si is not None and len(si.on_wait) > limit:
                    waits = list(si.on_wait)
                    keep = waits[-limit:]
                    for i, w in enumerate(waits[:-limit]):
                        nop = mybir.InstNoOp(
                            name=f"{ins.name}_w{i}",
                            engine=ins.engine,
                            sync_info=mybir.SyncInfo(on_wait=[w], on_update=[]),
                            bass_nofuse=True,
                        )
                        nc.register_instruction(nop, overwrite=True)
                        out.append(nop)
                    ins.sync_info = mybir.SyncInfo(
                        on_wait=keep, on_update=list(si.on_update)
                    )
                    changed = True
                out.append(ins)
            if changed:
                bb.instructions = out


def _build():
    import concourse.bass as bass
    import concourse.mybir as mybir
    import concourse.tile as tile
    from concourse.bass import ts

    dt = mybir.dt
    f32, bf16, f8 = dt.float32, dt.bfloat16, dt.float8e4
    Exp = mybir.ActivationFunctionType.Exp
    Ln = mybir.ActivationFunctionType.Ln
    Copy = mybir.ActivationFunctionType.Copy
    DR = mybir.MatmulPerfMode.DoubleRow
    EXP_SCALE = 0.125 / (WSCALE * WSCALE)

    nc = bass.Bass()
    # fp8 Q/K operands, host pre-interleaved: chunk-pair axes are (pc, i)
    # with input channel c = 128*(2*pc+i)+partition
    xf8_d = nc.dram_tensor("xf8", [128, NPC, 2, N_TOK], f8, kind="ExternalInput")
    xq8_d = nc.dram_tensor("xq8", [128, NPC, 2, TB], f8, kind="ExternalInput")
    wq2_d = nc.dram_tensor("wq2", [128, NPC, 2, C], f8, kind="ExternalInput")
    wk2_d = nc.dram_tensor("wk2", [128, NPC, 2, C], f8, kind="ExternalInput")
    # bf16 V/fc path
    xt_d = nc.dram_tensor("xt", [C, N_TOK], bf16, kind="ExternalInput")
    wvT_d = nc.dram_tensor("wvT", [128, NCH * C], bf16, kind="ExternalInput")
    wfT_d = nc.dram_tensor("wfT", [128, NCH * C], bf16, kind="ExternalInput")
    bias_d = nc.dram_tensor("bias", [128, 2 * NCH], f32, kind="ExternalInput")
    bfc_d = nc.dram_tensor("bfc", [1, C], bf16, kind="ExternalInput")
    out_d = nc.dram_tensor("out", [TB, C], bf16, kind="ExternalOutput")

    with tile.TileContext(nc) as tc:
        with (
            tc.tile_pool(name="wp", bufs=1) as wp,
            tc.tile_pool(name="data", bufs=1) as data,
            tc.tile_pool(name="ep", bufs=6) as ep,
            tc.tile_pool(name="np_", bufs=2) as npool,
            tc.tile_pool(name="scp", bufs=2, space=bass.MemorySpace.PSUM) as scp,
            tc.tile_pool(name="ap_", bufs=1, space=bass.MemorySpace.PSUM) as apool,
            tc.tile_pool(name="aux", bufs=2, space=bass.MemorySpace.PSUM) as aux,
        ):
            # ---- constants / weights ----
            wq2 = wp.tile([128, NPC, 2, C], f8, tag="wq", name="wq2")
            wk2 = wp.tile([128, NPC, 2, C], f8, tag="wk", name="wk2")
            wv_all = wp.tile([128, NCH * C], bf16, tag="wv", name="wv_all")
            wf_all = wp.tile([128, NCH * C], bf16, tag="wf", name="wf_all")
            wv = [wv_all[:, ts(c, C)] for c in range(NCH)]
            wf = [wf_all[:, ts(c, C)] for c in range(NCH)]
            bias_all = wp.tile([128, 2 * NCH], f32, tag="bias", name="bias_all")
            bqt = [bias_all[:, d : d + 1] for d in range(NCH)]
            bkt = [bias_all[:, NCH + d : NCH + d + 1] for d in range(NCH)]
            bfct = wp.tile([1, C], bf16, tag="bfct", name="bfct")
            ones_t = wp.tile([128, TB], bf16, tag="ones", name="ones_t")
            nc.vector.memset(ones_t[:], 1.0)
            ones_f = wp.tile([128, 64], f32, tag="onesf", name="ones_f")
            nc.vector.memset(ones_f[:], 1.0)

            # preload the Exp ACT table during the input-DMA window
            actwarm = npool.tile([1, 8], f32, tag="actw", name="actwarm")
            nc.scalar.activation(out=actwarm[:], in_=ones_f[0:1, 0:8], func=Exp)

            # PE warmup: dummy matmuls ramp the HAM activity monitor through
            # the input-load window so the projections run at speed
            warm = aux.tile([128, TB], f32, tag="aux", name="warm")
            for r in range(8):
                nc.tensor.matmul(
                    warm[:], ones_t[0:1, 0:128], ones_t[0:1, :],
                    start=(r == 0), stop=(r == 7),
                )

            # ---- activations in ----
            xf8 = data.tile([128, NPC, 2, N_TOK], f8, tag="xf8", name="xf8")
            xq8 = data.tile([128, NPC, 2, TB], f8, tag="xq8", name="xq8")
            xt = [
                data.tile([128, N_TOK], bf16, tag=f"xt{c}", name=f"xt{c}")
                for c in range(NCH)
            ]

            # ---- input DMAs. Time-to-first-exp is bound by bias+wk2+
            # xf8(jb0)+xq8+wq2; xt/wv (V path) are due by pair0 t=2 ----
            nc.sync.dma_start(out=bias_all[:], in_=bias_d[:])
            nc.scalar.dma_start(out=wq2[:], in_=wq2_d[:])
            nc.sync.dma_start(out=wk2[:], in_=wk2_d[:])
            nc.scalar.dma_start(out=xq8[:], in_=xq8_d[:])
            nc.sync.dma_start(out=xf8[:, :, :, ts(0, TB)], in_=xf8_d[:, :, :, ts(0, TB)])
            nc.scalar.dma_start(out=xt[0][:], in_=xt_d[ts(0, 128), :])
            nc.sync.dma_start(
                out=xf8[:, :, :, TB:N_TOK], in_=xf8_d[:, :, :, TB:N_TOK]
            )
            nc.scalar.dma_start(out=xt[1][:], in_=xt_d[ts(1, 128), :])
            nc.gpsimd.dma_start(out=xt[2][:], in_=xt_d[ts(2, 128), :])
            nc.gpsimd.dma_start(out=xt[3][:], in_=xt_d[ts(3, 128), :])
            nc.gpsimd.dma_start(out=wv_all[:], in_=wvT_d[:])
            nc.gpsimd.dma_start(out=wf_all[:], in_=wfT_d[:])
            nc.gpsimd.dma_start(out=bfct[:], in_=bfc_d[:])

            # ---- persistent intermediates ----
            kt8 = [
                data.tile([128, N_TOK], f8, tag=f"kt8_{d}", name=f"kt8_{d}")
                for d in range(NCH)
            ]
            qt8 = [
                data.tile([128, TB], f8, tag=f"qt8_{d}", name=f"qt8_{d}")
                for d in range(NCH)
            ]
            # V pairs: [128p tok, half i, head, dk+ones+pad] -- the pad
            # column keeps the DoubleRow Ldweights length even
            vpad2 = [
                data.tile([128, 2, HEADS, DK + 2], f8, tag=f"vp{t}", name=f"vp{t}")
                for t in range(NT)
            ]
            att = [
                data.tile([128, TB], bf16, tag=f"att{c}", name=f"att{c}")
                for c in range(NCH)
            ]
            for t in range(NT):
                nc.vector.memset(vpad2[t][:, :, :, DK : DK + 2], 0.0)
                nc.vector.memset(vpad2[t][:, :, :, DK : DK + 1], 1.0)

            def proj_q(d):
                """Q^T d-tile (128 chans = heads 2d, 2d+1) + bias -> fp8."""
                qp = aux.tile([128, TB], f32, tag="aux", name=f"qp{d}")
                for pc in range(NPC):
                    nc.tensor.matmul(
                        qp[:], wq2[:, pc, :, ts(d, 128)], xq8[:, pc],
                        start=(pc == 0), stop=(pc == NPC - 1), perf_mode=DR,
                    )
                nc.vector.tensor_scalar_add(out=qt8[d][:], in0=qp[:], scalar1=bqt[d])

            def proj_k(d, jb):
                """K^T d-tile, token block jb + bias -> fp8."""
                kp = aux.tile([128, TB], f32, tag="aux", name=f"kp{d}_{jb}")
                for pc in range(NPC):
                    nc.tensor.matmul(
                        kp[:], wk2[:, pc, :, ts(d, 128)], xf8[:, pc, :, ts(jb, TB)],
                        start=(pc == 0), stop=(pc == NPC - 1), perf_mode=DR,
                    )
                nc.vector.tensor_scalar_add(
                    out=kt8[d][:, ts(jb, TB)], in0=kp[:], scalar1=bkt[d]
                )

            def proj_v(j):
                """V j-tile (bf16 matmuls) -> vpad2[j//2] half j%2, fp8."""
                vp = aux.tile([128, C], f32, tag="aux", name=f"vpp{j}")
                for c in range(NCH):
                    nc.tensor.matmul(
                        vp[:], xt[c][:, ts(j, 128)], wv[c],
                        start=(c == 0), stop=(c == NCH - 1),
                    )
                nc.vector.tensor_copy(
                    out=vpad2[j // 2][:, j % 2, :, 0:DK],
                    in_=vp[:].rearrange("p (h d) -> p h d", h=HEADS),
                )

            def scores_exp(p, t, hh):
                """Scores for head 2p+hh over j-tiles 2t,2t+1 -> exp -> fp8 e.
                dk=64 contraction straight off the fp8 projection outputs;
                the PE overlaps the back-to-back stationary loads."""
                sc = scp.tile([128, 2, TB], f32, tag="sc", name=f"sc{p}_{t}_{hh}")
                for i2 in range(2):
                    nc.tensor.matmul(
                        sc[:, i2, :],
                        kt8[p][ts(hh, 64), ts(2 * t + i2, 128)],
                        qt8[p][ts(hh, 64), :],
                    )
                e = ep.tile([128, 2, TB], f8, tag="e", name=f"e{p}_{t}_{hh}")
                nc.scalar.activation(out=e[:], in_=sc[:], func=Exp, scale=EXP_SCALE)
                return e

            def attn_v(p, t, hh, e, a):
                nc.tensor.matmul(
                    a[:], vpad2[t][:, :, 2 * p + hh, :], e[:],
                    start=(t == 0), stop=(t == NT - 1), perf_mode=DR,
                )

            def normalize(pp, a_sb, hh, rb=None):
                """Softmax normalization for pair pp's head hh. Denominator
                reciprocal on DVE; broadcast over the 64 dk partitions via a
                PE ones-matmul; scale on DVE."""
                if rb is None:
                    rb = aux.tile([64, TB], f32, tag="aux", name=f"rb{pp}_{hh}")[:]
                rcp = npool.tile([65, TB], f32, tag="rcp", bufs=4, name=f"rcp{pp}_{hh}")
                nc.vector.reciprocal(out=rcp[64:65, :], in_=a_sb[64:65, :])
                nc.tensor.matmul(rb, ones_f[64:65, :], rcp[64:65, :])
                nc.vector.tensor_mul(
                    out=att[pp][ts(hh, 64), :], in0=a_sb[0:64, :], in1=rb
                )

            def fc_prefill(tt, fp):
                nc.tensor.matmul(
                    fp, ones_t[0:1, 0:128], bfct[:], start=True, stop=False
                )
                for c in range(NCH - 1):
                    nc.tensor.matmul(
                        fp, att[c][:, ts(tt, 128)], wf[c],
                        start=False, stop=False,
                    )

            # ---- main pipeline ----
            fcs = {}
            proj_q(0)
            proj_k(0, 0)
            prev = None  # previous pair's SBUF accumulator copies
            pending = None  # previous pair's boundary work, run at (p, t=0)
            for p in range(NCH):  # head pair p = heads 2p, 2p+1
                a0 = apool.tile([DK + 2, TB], f32, tag="a0", name=f"a0_{p}")
                a1 = apool.tile([DK + 2, TB], f32, tag="a1", name=f"a1_{p}")
                es = {}
                for t in range(NT):
                    # pair seam: emit a two-step scores runway before the
                    # previous pair's boundary leftovers, so the exp stream
                    # never waits on the PE's in-order queue
                    if not (p >= 1 and t == 1):
                        es[(t, 0)] = scores_exp(p, t, 0)
                        es[(t, 1)] = scores_exp(p, t, 1)
                    if p >= 1 and t == 0:
                        es[(1, 0)] = scores_exp(p, 1, 0)
                        es[(1, 1)] = scores_exp(p, 1, 1)
                        prev = pending()
                        pending = None
                    # pair 0: remaining K blocks paced with the xf8 DMA and
                    # kept off the PE-tight t=1 step
                    if p == 0 and t in (0, 2, 3):
                        proj_k(0, 1 if t == 0 else t)
                    # next pair's K/Q projections, spread mid-pair (pair 0
                    # is PE-tight: its share shifts later and projk(1,3)
                    # rides the seam leftover block)
                    if p == 0:
                        if t == 4:
                            proj_q(1)
                        elif t >= 5:
                            proj_k(1, t - 5)
                    elif p < NCH - 1:
                        if t == 3:
                            proj_q(p + 1)
                        elif t >= 4:
                            proj_k(p + 1, t - 4)
                    else:
                        # last pair: fc pre-accumulation (bias + chunks 0..2)
                        # as PE filler; fp0/fp1 in the freed aux slots
                        if t in (3, 4):
                            fcs[t - 3] = aux.tile(
                                [128, C], f32, tag="aux", name=f"fp{t - 3}"
                            )[:]
                            fc_prefill(t - 3, fcs[t - 3])
                    # previous pair's normalization, deferred into this pair
                    if prev is not None and t in (1, 2):
                        hh = t - 1
                        normalize(p - 1, prev[hh], hh)
                    # attnV: pair 0 lags two t-steps so the xt/wv DMAs and
                    # the V projections stay off the exp-stream critical path
                    tv = t - 2 if p == 0 else t
                    if 0 <= tv:
                        if p == 0:
                            proj_v(2 * tv)
                            proj_v(2 * tv + 1)
                        attn_v(p, tv, 0, es.pop((tv, 0)), a0)
                        attn_v(p, tv, 1, es.pop((tv, 1)), a1)

                def boundary(p=p, a0=a0, a1=a1, es=es):
                    if p == 0:
                        for tv in (NT - 2, NT - 1):
                            proj_v(2 * tv)
                            proj_v(2 * tv + 1)
                            attn_v(p, tv, 0, es.pop((tv, 0)), a0)
                            attn_v(p, tv, 1, es.pop((tv, 1)), a1)
                        proj_k(1, 3)
                    # evacuate accumulators to SBUF (DVE) so the banks free
                    a_sb0 = npool.tile(
                        [DK + 1, TB], f32, tag="asb", bufs=4, name=f"asb0_{p}"
                    )
                    a_sb1 = npool.tile(
                        [DK + 1, TB], f32, tag="asb", bufs=4, name=f"asb1_{p}"
                    )
                    nc.vector.tensor_copy(out=a_sb0[:], in_=a0[0 : DK + 1, :])
                    nc.vector.tensor_copy(out=a_sb1[:], in_=a1[0 : DK + 1, :])
                    return (a_sb0, a_sb1)

                if p < NCH - 1:
                    pending = boundary
                else:
                    # last pair: normalize straight out of PSUM at the tail
                    prev = (a0, a1)

            # ---- tail: only the final fc chunk waits on normalize(3).
            # a0/a1 stay live (PSUM-direct reciprocal); head 6's reciprocal
            # runs on ACT (idle now; Ln/Exp/Copy share the loaded table),
            # head 7's on DVE, in parallel. The freed a0/a1 slots take the
            # rb broadcasts; fp2/fp3 take the scores slots ----
            a0, a1 = prev
            fp2 = scp.tile([128, 2, C], f32, tag="sc", name="fp2")
            fp3 = scp.tile([128, 2, C], f32, tag="sc", name="fp3")
            fcs[2], fcs[3] = fp2[:, 0, :], fp3[:, 0, :]
            fc_prefill(2, fcs[2])
            fc_prefill(3, fcs[3])

            lnt0 = npool.tile([65, TB], f32, tag="rcp", bufs=4, name="lnt3_0")
            rcp0 = npool.tile([65, TB], f32, tag="rcp", bufs=4, name="rcp3_0")
            rcp1 = npool.tile([65, TB], f32, tag="rcp", bufs=4, name="rcp3_1")
            asb0 = npool.tile([DK + 1, TB], f32, tag="asb", bufs=4, name="asb3_0")
            asb1 = npool.tile([DK + 1, TB], f32, tag="asb", bufs=4, name="asb3_1")
            nc.scalar.activation(out=lnt0[64:65, :], in_=a0[64:65, :], func=Ln)
            nc.vector.reciprocal(out=rcp1[64:65, :], in_=a1[64:65, :])
            nc.scalar.activation(
                out=rcp0[64:65, :], in_=lnt0[64:65, :], func=Exp, scale=-1.0
            )
            nc.scalar.activation(out=asb0[:], in_=a0[0 : DK + 1, :], func=Copy)
            nc.vector.tensor_copy(out=asb1[:], in_=a1[0 : DK + 1, :])
            rb0 = apool.tile([64, TB], f32, tag="a0", name="rb3_0")
            rb1 = apool.tile([64, TB], f32, tag="a1", name="rb3_1")
            nc.tensor.matmul(rb0[:], ones_f[64:65, :], rcp0[64:65, :])
            nc.vector.tensor_mul(
                out=att[3][0:64, :], in0=asb0[0:64, :], in1=rb0[:]
            )
            nc.tensor.matmul(rb1[:], ones_f[64:65, :], rcp1[64:65, :])
            nc.vector.tensor_mul(
                out=att[3][64:128, :], in0=asb1[0:64, :], in1=rb1[:]
            )

            # final fc split by head: the head-6 halves run as soon as the
            # first scale lands, in parallel with head 7's normalization
            for tt in range(NTT):
                nc.tensor.matmul(
                    fcs[tt], att[3][0:64, ts(tt, 128)], wf[3][0:64, :],
                    start=False, stop=False,
                )
            for tt in range(NTT):
                nc.tensor.matmul(
                    fcs[tt], att[3][64:128, ts(tt, 128)], wf[3][64:128, :],
                    start=False, stop=True,
                )
                # evacuate on ACT (idle at the tail; Copy shares Exp's table)
                ot = npool.tile([128, C], bf16, tag="ot", bufs=4, name=f"ot{tt}")
                nc.scalar.activation(out=ot[:], in_=fcs[tt], func=Copy)
                (nc.sync if tt % 2 == 0 else nc.gpsimd).dma_start(
                    out=out_d[ts(tt, 128), :], in_=ot[:]
                )

    _split_multi_waits(nc)
    nc.finalize()
    return nc


def get_nc():
    if "nc" not in _CACHE:
        _install_tile_drain_patch()
        _CACHE["nc"] = _build()
    return _CACHE["nc"]


def make_in_maps(x, Wq, bq, Wk, bk, Wv, bv, Wfc, bfc):
    bf = ml_dtypes.bfloat16
    f8 = ml_dtypes.float8_e4m3
    x = np.asarray(x, np.float32)
    Wq, Wk, Wv, Wfc = (np.asarray(w, np.float32) for w in (Wq, Wk, Wv, Wfc))
    bq, bk, bv, bfc = (np.asarray(v, np.float32) for v in (bq, bk, bv, bfc))
    S = np.float32(WSCALE)

    def interleave(wT):
        # [C, cols] -> [128, NCH*cols] with chunk c at columns [c*cols:...]
        cols = wT.shape[1]
        return np.ascontiguousarray(
            wT.reshape(NCH, 128, cols).transpose(1, 0, 2).reshape(128, NCH * cols)
        )

    def dr_pack(m):
        # [C, cols] -> [128, NPC, 2, cols]; input channel 128*(2pc+i)+p
        cols = m.shape[1]
        return np.ascontiguousarray(m.reshape(NPC, 2, 128, cols).transpose(2, 0, 1, 3))

    bfc_folded = (Wfc @ bv + bfc).reshape(1, C).astype(bf)
    wq2 = dr_pack((np.ascontiguousarray(Wq.T) * S).astype(f8))
    wk2 = dr_pack((np.ascontiguousarray(Wk.T) * S).astype(f8))
    wvT = interleave((np.ascontiguousarray(Wv.T) * S).astype(bf))
    wfT = interleave(np.ascontiguousarray((Wfc / S).T).astype(bf))
    bias_c = (
        np.concatenate([bq.reshape(NCH, 128).T, bk.reshape(NCH, 128).T], axis=1) * S
    ).astype(np.float32)

    in_maps = []
    for core in range(N_CORES):
        b, t = divmod(core, N_TOK // TB)
        XT = np.ascontiguousarray(x[b].reshape(N_TOK, C).T)
        in_maps.append(
            {
                "xf8": dr_pack(XT.astype(f8)),
                "xq8": dr_pack(
                    np.ascontiguousarray(XT[:, t * TB : (t + 1) * TB]).astype(f8)
                ),
                "wq2": wq2,
                "wk2": wk2,
                "xt": XT.astype(bf),
                "wvT": wvT,
                "wfT": wfT,
                "bias": bias_c,
                "bfc": bfc_folded,
            }
        )
    return in_maps


def assemble(outs):
    """outs: list of 8 dicts with 'out' (512, 512) -> (2, 512, 64, 32)."""
    per_batch = [
        np.concatenate([outs[b * 4 + t]["out"] for t in range(4)], axis=0)
        for b in range(B)
    ]
    return np.stack(per_batch).reshape(B, C, 64, 32).astype(np.float32)


def kernel(**inputs):
    from concourse.bass_utils import run_bass_kernel_spmd

    nc = get_nc()
    in_maps = make_in_maps(**inputs)
    res = run_bass_kernel_spmd(nc, in_maps, list(range(N_CORES)))
    return assemble(res.results)
